# revision 2
# baseline (speedup 1.0000x reference)
"""Trainium2 Bass kernel for the ragged triangular-GEMM decoder.

Computation (reference): out[b, i, :] = sum_{l<=i} x[b, l, :] @ W_i[l]
with x: [128, 12, 4096] fp32, W_i: [(i+1), 4096, 768] fp32, out: [128, 12, 768].

Sharding: tensor-parallel over the output feature dim A=768 -> 96 per core.
Each core reads all of x (25 MB) and a 96-wide slice of every W_i
(~123 MB) -- weight traffic is the roofline and is split perfectly 8-way.

Per-core kernel: loop over source layer l (0..11). x's l-th chunk
contributes to output layers i >= l, i.e. columns [l*96, 1152) of a
[128b x 1152] accumulator that lives in 3 PSUM banks (384 fp32 each)
for the whole kernel. For each l the contraction over f=4096 runs as 32
k-chunks of 128; the moving operand is the concatenation over i>=l of
W_i[l]'s 96-col slice, split at PSUM bank boundaries.
"""

import numpy as np
from contextlib import ExitStack

import concourse.bass as bass
import concourse.tile as tile
from concourse import bacc, mybir
from concourse.bass_utils import run_bass_kernel_spmd

N_CORES = 8
B = 128
L = 12
F = 4096
A = 768
ASL = A // N_CORES          # 96 cols per core
KK = F // 128               # 32 k-chunks per layer
OUT_W = L * ASL             # 1152 accumulator cols
BANK_W = 384                # psum bank chunk width (<=512 fp32)
N_BANKS = OUT_W // BANK_W   # 3

# kk-group sizing: keep each W DMA's per-partition chunk <= ~40 KB
_W_TILE_BYTES = 40000

_compiled_nc = None


def _kk_groups(w_cols: int) -> list[tuple[int, int]]:
    bytes_per_kk = w_cols * 4
    n_groups = max(1, -(-KK * bytes_per_kk // _W_TILE_BYTES))
    kg = -(-KK // n_groups)
    out = []
    s = 0
    while s < KK:
        e = min(KK, s + kg)
        out.append((s, e))
        s = e
    return out


def _segments(l: int) -> list[tuple[int, int, int]]:
    """Matmul col segments for source layer l: (bank, c0, c1) global cols."""
    segs = []
    start = l * ASL
    for b in range(N_BANKS):
        b0, b1 = b * BANK_W, (b + 1) * BANK_W
        c0 = max(start, b0)
        if c0 < b1:
            segs.append((b, c0, b1))
    return segs


def _build():
    nc = bacc.Bacc("TRN2", target_bir_lowering=False, debug=False,
                   num_devices=N_CORES)

    xt_d = nc.dram_tensor("xt", [L, 128, KK, B], mybir.dt.float32,
                          kind="ExternalInput").ap()
    w_d = [
        nc.dram_tensor(f"w{l}", [128, KK, (L - l) * ASL], mybir.dt.float32,
                       kind="ExternalInput").ap()
        for l in range(L)
    ]
    out_d = nc.dram_tensor("out", [B, OUT_W], mybir.dt.float32,
                           kind="ExternalOutput").ap()

    # last (l, kk) writing each bank -> stop flag
    last_l_for_bank = {b: max(l for l in range(L)
                              for bb, _, _ in _segments(l) if bb == b)
                      for b in range(N_BANKS)}

    with tile.TileContext(nc) as tc:
        with ExitStack() as ctx:
            xpool = ctx.enter_context(tc.tile_pool(name="x", bufs=2))
            wpool = ctx.enter_context(tc.tile_pool(name="w", bufs=3))
            opool = ctx.enter_context(tc.tile_pool(name="o", bufs=1))
            ppool = ctx.enter_context(tc.tile_pool(name="ps", bufs=1,
                                                   space="PSUM"))

            ps = [ppool.tile([B, BANK_W], mybir.dt.float32, tag=f"ps{b}",
                             name=f"ps{b}")
                  for b in range(N_BANKS)]
            started = [False] * N_BANKS

            # max W tile free size, shared tag so slots are max-sized
            for l in range(L):
                w_cols = (L - l) * ASL
                xl = xpool.tile([128, KK, B], mybir.dt.float32, tag="xl")
                nc.scalar.dma_start(xl[:], xt_d[l])

                for (g0, g1) in _kk_groups(w_cols):
                    wg = wpool.tile([128, g1 - g0, w_cols],
                                    mybir.dt.float32, tag="wg")
                    nc.sync.dma_start(wg[:], w_d[l][:, g0:g1, :])

                    for kk in range(g0, g1):
                        for (b, c0, c1) in _segments(l):
                            start = not started[b]
                            started[b] = True
                            stop = (l == last_l_for_bank[b]) and (kk == KK - 1)
                            nc.tensor.matmul(
                                ps[b][:, c0 - b * BANK_W:c1 - b * BANK_W],
                                xl[:, kk, :],
                                wg[:, kk - g0, c0 - l * ASL:c1 - l * ASL],
                                start=start, stop=stop,
                            )

            ot = opool.tile([B, OUT_W], mybir.dt.float32)
            for b in range(N_BANKS):
                nc.vector.tensor_copy(ot[:, b * BANK_W:(b + 1) * BANK_W],
                                      ps[b][:])
            nc.sync.dma_start(out_d[:], ot[:])

    nc.compile()
    return nc


def _pack_inputs(x: np.ndarray, Ws: list[np.ndarray]):
    """Host-side shard + layout. Returns in_maps (one dict per core)."""
    # xt[l, p, kk, b] = x[b, l, kk*128 + p]  (same for every core)
    xt = np.ascontiguousarray(
        x.reshape(B, L, KK, 128).transpose(1, 3, 2, 0))
    # per (l): concat over i >= l of W_i[l]'s per-core 96-col slice
    # wslice[i][l] -> [cores, 128p, KK, 96]
    w_per_l = []
    for l in range(L):
        parts = []
        for i in range(l, L):
            wl = Ws[i][l]  # [F, A]
            parts.append(wl.reshape(KK, 128, N_CORES, ASL)
                           .transpose(2, 1, 0, 3))  # [cores, 128, KK, 96]
        w_per_l.append(np.concatenate(parts, axis=3))  # [cores,128,KK,(L-l)*96]

    in_maps = []
    for j in range(N_CORES):
        m = {"xt": xt}
        for l in range(L):
            m[f"w{l}"] = np.ascontiguousarray(w_per_l[l][j])
        in_maps.append(m)
    return in_maps


def _run(inputs: dict, trace: bool = False):
    global _compiled_nc
    if _compiled_nc is None:
        _compiled_nc = _build()
    x = np.asarray(inputs["x"], dtype=np.float32)
    Ws = [np.asarray(inputs[f"W_{i}"], dtype=np.float32) for i in range(L)]
    in_maps = _pack_inputs(x, Ws)
    res = run_bass_kernel_spmd(_compiled_nc, in_maps,
                               core_ids=list(range(N_CORES)), trace=trace)
    out = np.empty((B, L, A), dtype=np.float32)
    for j in range(N_CORES):
        out[:, :, j * ASL:(j + 1) * ASL] = res.results[j]["out"].reshape(
            B, L, ASL)
    return out, res


def kernel(**inputs: np.ndarray) -> np.ndarray:
    out, _ = _run(inputs, trace=False)
    return out


# revision 3
# speedup vs baseline: 1.1420x; 1.1420x over previous
"""Trainium2 Bass kernel for the ragged triangular-GEMM decoder.

Computation (reference): out[b, i, :] = sum_{l<=i} x[b, l, :] @ W_i[l]
with x: [128, 12, 4096] fp32, W_i: [(i+1), 4096, 768] fp32, out: [128, 12, 768].

Sharding: tensor-parallel over the output feature dim A=768 -> 96 per core.
Each core reads all of x (25 MB) and a 96-wide slice of every W_i
(~123 MB) -- weight traffic is the roofline and is split perfectly 8-way.

Per-core kernel: loop over source layer l (0..11). x's l-th chunk
contributes to output layers i >= l, i.e. columns [l*96, 1152) of a
[128b x 1152] accumulator that lives in 3 PSUM banks (384 fp32 each)
for the whole kernel. For each l the contraction over f=4096 runs as 32
k-chunks of 128; the moving operand is the concatenation over i>=l of
W_i[l]'s 96-col slice, split at PSUM bank boundaries.

Matmuls run in fp32r (fp32 rounded to 11 explicit mantissa bits,
hardware-verified round-to-nearest-even) -- 4x the fp32 matmul rate.
Operands are pre-rounded on the host so the loads stay on the fast
HWDGE rings; this is bitwise identical to the on-chip fp32->fp32r cast.
"""

import numpy as np
from contextlib import ExitStack

import concourse.bass as bass
import concourse.tile as tile
from concourse import bacc, mybir
from concourse.bass_utils import run_bass_kernel_spmd

N_CORES = 8
B = 128
L = 12
F = 4096
A = 768
ASL = A // N_CORES          # 96 cols per core
KK = F // 128               # 32 k-chunks per layer
OUT_W = L * ASL             # 1152 accumulator cols
BANK_W = 384                # psum bank chunk width (<=512 fp32)
N_BANKS = OUT_W // BANK_W   # 3

# kk-group sizing: keep each W DMA's per-partition chunk <= ~40 KB
_W_TILE_BYTES = 40000

_compiled_nc = None


def _kk_groups(w_cols: int) -> list[tuple[int, int]]:
    bytes_per_kk = w_cols * 4
    n_groups = max(1, -(-KK * bytes_per_kk // _W_TILE_BYTES))
    kg = -(-KK // n_groups)
    out = []
    s = 0
    while s < KK:
        e = min(KK, s + kg)
        out.append((s, e))
        s = e
    return out


def _segments(l: int) -> list[tuple[int, int, int]]:
    """Matmul col segments for source layer l: (bank, c0, c1) global cols."""
    segs = []
    start = l * ASL
    for b in range(N_BANKS):
        b0, b1 = b * BANK_W, (b + 1) * BANK_W
        c0 = max(start, b0)
        if c0 < b1:
            segs.append((b, c0, b1))
    return segs


def _build():
    nc = bacc.Bacc("TRN2", target_bir_lowering=False, debug=False,
                   num_devices=N_CORES)

    rdt = mybir.dt.float32r
    xt_d = nc.dram_tensor("xt", [L, 128, KK, B], rdt,
                          kind="ExternalInput").ap()
    w_d = [
        nc.dram_tensor(f"w{l}", [128, KK, (L - l) * ASL], rdt,
                       kind="ExternalInput").ap()
        for l in range(L)
    ]
    out_d = nc.dram_tensor("out", [B, OUT_W], mybir.dt.float32,
                           kind="ExternalOutput").ap()

    # last (l, kk) writing each bank -> stop flag
    last_l_for_bank = {b: max(l for l in range(L)
                              for bb, _, _ in _segments(l) if bb == b)
                      for b in range(N_BANKS)}

    # alternate big loads across the two HWDGE rings
    rings = [nc.sync, nc.scalar]
    ring_i = 0

    def next_ring():
        nonlocal ring_i
        r = rings[ring_i % 2]
        ring_i += 1
        return r

    with tile.TileContext(nc) as tc:
        with ExitStack() as ctx:
            xpool = ctx.enter_context(tc.tile_pool(name="x", bufs=2))
            wpool = ctx.enter_context(tc.tile_pool(name="w", bufs=3))
            opool = ctx.enter_context(tc.tile_pool(name="o", bufs=1))
            ppool = ctx.enter_context(tc.tile_pool(name="ps", bufs=1,
                                                   space="PSUM"))

            ps = [ppool.tile([B, BANK_W], mybir.dt.float32, tag=f"ps{b}",
                             name=f"ps{b}")
                  for b in range(N_BANKS)]
            started = [False] * N_BANKS

            for l in range(L):
                w_cols = (L - l) * ASL
                xl = xpool.tile([128, KK, B], rdt, tag="xl")
                next_ring().dma_start(xl[:], xt_d[l])

                for (g0, g1) in _kk_groups(w_cols):
                    wg = wpool.tile([128, g1 - g0, w_cols], rdt, tag="wg")
                    next_ring().dma_start(wg[:], w_d[l][:, g0:g1, :])

                    for kk in range(g0, g1):
                        for (b, c0, c1) in _segments(l):
                            start = not started[b]
                            started[b] = True
                            stop = (l == last_l_for_bank[b]) and (kk == KK - 1)
                            nc.tensor.matmul(
                                ps[b][:, c0 - b * BANK_W:c1 - b * BANK_W],
                                xl[:, kk, :],
                                wg[:, kk - g0, c0 - l * ASL:c1 - l * ASL],
                                start=start, stop=stop,
                            )

            ot = opool.tile([B, OUT_W], mybir.dt.float32)
            for b in range(N_BANKS):
                nc.vector.tensor_copy(ot[:, b * BANK_W:(b + 1) * BANK_W],
                                      ps[b][:])
            nc.sync.dma_start(out_d[:], ot[:])

    nc.compile()
    return nc


def _round_fp32r(a: np.ndarray) -> np.ndarray:
    """Round fp32 to 11 explicit mantissa bits, nearest-even (= HW fp32r)."""
    u = np.ascontiguousarray(a, dtype=np.float32).view(np.uint32)
    low = (u >> np.uint32(12)) & np.uint32(1)
    u = (u + np.uint32(0x7FF) + low) & np.uint32(0xFFFFF000)
    return u.view(np.float32)


def _pack_inputs(x: np.ndarray, Ws: list[np.ndarray]):
    """Host-side shard + layout. Returns in_maps (one dict per core)."""
    # xt[l, p, kk, b] = x[b, l, kk*128 + p]  (same for every core)
    xt = _round_fp32r(np.ascontiguousarray(
        x.reshape(B, L, KK, 128).transpose(1, 3, 2, 0)))
    # per (l): concat over i >= l of W_i[l]'s per-core 96-col slice
    w_per_l = []
    for l in range(L):
        parts = []
        for i in range(l, L):
            wl = Ws[i][l]  # [F, A]
            parts.append(wl.reshape(KK, 128, N_CORES, ASL)
                           .transpose(2, 1, 0, 3))  # [cores, 128, KK, 96]
        w_per_l.append(np.concatenate(parts, axis=3))  # [cores,128,KK,(L-l)*96]

    in_maps = []
    for j in range(N_CORES):
        m = {"xt": xt}
        for l in range(L):
            m[f"w{l}"] = _round_fp32r(np.ascontiguousarray(w_per_l[l][j]))
        in_maps.append(m)
    return in_maps


def _run(inputs: dict, trace: bool = False):
    global _compiled_nc
    if _compiled_nc is None:
        _compiled_nc = _build()
    x = np.asarray(inputs["x"], dtype=np.float32)
    Ws = [np.asarray(inputs[f"W_{i}"], dtype=np.float32) for i in range(L)]
    in_maps = _pack_inputs(x, Ws)
    res = run_bass_kernel_spmd(_compiled_nc, in_maps,
                               core_ids=list(range(N_CORES)), trace=trace)
    out = np.empty((B, L, A), dtype=np.float32)
    for j in range(N_CORES):
        out[:, :, j * ASL:(j + 1) * ASL] = res.results[j]["out"].reshape(
            B, L, ASL)
    return out, res


def kernel(**inputs: np.ndarray) -> np.ndarray:
    out, _ = _run(inputs, trace=False)
    return out


# revision 4
# speedup vs baseline: 2.2599x; 1.9788x over previous
"""Trainium2 Bass kernel for the ragged triangular-GEMM decoder.

Computation (reference): out[b, i, :] = sum_{l<=i} x[b, l, :] @ W_i[l]
with x: [128, 12, 4096] fp32, W_i: [(i+1), 4096, 768] fp32, out: [128, 12, 768].

Sharding: tensor-parallel over the output feature dim A=768 -> 96 per core.
Each core reads all of x and a 96-wide slice of every W_i -- weight
traffic dominates and is split perfectly 8-way.

Per-core kernel: loop over source layer l (0..11). x's l-th chunk
contributes to output layers i >= l, i.e. columns [l*96, 1152) of a
[128b x 1152] accumulator that lives in 3 PSUM banks (384 fp32 each)
for the whole kernel. For each l the contraction over f=4096 runs as 32
k-chunks of 128; the moving operand is the concatenation over i>=l of
W_i[l]'s 96-col slice, split at PSUM bank boundaries.

MM_MODE picks the operand precision (accumulation is always fp32 PSUM):
  "f16"  -- operands cast to fp16 on host. 10+1-bit mantissa, half the
            HBM traffic of fp32; matmul at full PE rate. Default.
  "f32r" -- fp32 rounded to 11+1 mantissa bits on host (bitwise equal to
            the HW fp32r cast), full fp32 traffic, 1 cyc/row for >=256-col
            moving operands.
  "f32"  -- exact fp32, 4 cyc/row matmuls.
"""

import numpy as np
from contextlib import ExitStack

import concourse.bass as bass
import concourse.tile as tile
from concourse import bacc, mybir
from concourse.bass_utils import run_bass_kernel_spmd

MM_MODE = "f16"

N_CORES = 8
B = 128
L = 12
F = 4096
A = 768
ASL = A // N_CORES          # 96 cols per core
KK = F // 128               # 32 k-chunks per layer
OUT_W = L * ASL             # 1152 accumulator cols
BANK_W = 384                # psum bank chunk width (<=512 fp32)
N_BANKS = OUT_W // BANK_W   # 3

# kk-group sizing: keep each W DMA's per-partition chunk <= ~36 KB
_W_TILE_BYTES = 36000

_compiled_nc = None


def _mm_dtypes():
    if MM_MODE == "f16":
        return mybir.dt.float16, np.float16
    if MM_MODE == "f32r":
        return mybir.dt.float32r, np.float32
    return mybir.dt.float32, np.float32


def _kk_groups(w_cols: int, itemsize: int) -> list[tuple[int, int]]:
    bytes_per_kk = w_cols * itemsize
    n_groups = max(1, -(-KK * bytes_per_kk // _W_TILE_BYTES))
    kg = -(-KK // n_groups)
    out = []
    s = 0
    while s < KK:
        e = min(KK, s + kg)
        out.append((s, e))
        s = e
    return out


def _segments(l: int) -> list[tuple[int, int, int]]:
    """Matmul col segments for source layer l: (bank, c0, c1) global cols."""
    segs = []
    start = l * ASL
    for b in range(N_BANKS):
        b0, b1 = b * BANK_W, (b + 1) * BANK_W
        c0 = max(start, b0)
        if c0 < b1:
            segs.append((b, c0, b1))
    return segs


def _build():
    nc = bacc.Bacc("TRN2", target_bir_lowering=False, debug=False,
                   num_devices=N_CORES)

    rdt, npdt = _mm_dtypes()
    isz = np.dtype(npdt).itemsize
    xt_d = nc.dram_tensor("xt", [L, 128, KK, B], rdt,
                          kind="ExternalInput").ap()
    w_d = [
        nc.dram_tensor(f"w{l}", [128, KK, (L - l) * ASL], rdt,
                       kind="ExternalInput").ap()
        for l in range(L)
    ]
    out_d = nc.dram_tensor("out", [B, OUT_W], mybir.dt.float32,
                           kind="ExternalOutput").ap()

    # last (l, kk) writing each bank -> stop flag
    last_l_for_bank = {b: max(l for l in range(L)
                              for bb, _, _ in _segments(l) if bb == b)
                      for b in range(N_BANKS)}

    # alternate big loads across the two HWDGE rings
    rings = [nc.sync, nc.scalar]
    ring_i = 0

    def next_ring():
        nonlocal ring_i
        r = rings[ring_i % 2]
        ring_i += 1
        return r

    with tile.TileContext(nc) as tc:
        with ExitStack() as ctx:
            xpool = ctx.enter_context(tc.tile_pool(name="x", bufs=2))
            wpool = ctx.enter_context(tc.tile_pool(name="w", bufs=4))
            opool = ctx.enter_context(tc.tile_pool(name="o", bufs=1))
            ppool = ctx.enter_context(tc.tile_pool(name="ps", bufs=1,
                                                   space="PSUM"))

            ps = [ppool.tile([B, BANK_W], mybir.dt.float32, tag=f"ps{b}",
                             name=f"ps{b}")
                  for b in range(N_BANKS)]
            started = [False] * N_BANKS

            for l in range(L):
                w_cols = (L - l) * ASL
                xl = xpool.tile([128, KK, B], rdt, tag="xl")
                next_ring().dma_start(xl[:], xt_d[l])

                for (g0, g1) in _kk_groups(w_cols, isz):
                    wg = wpool.tile([128, g1 - g0, w_cols], rdt, tag="wg")
                    next_ring().dma_start(wg[:], w_d[l][:, g0:g1, :])

                    for kk in range(g0, g1):
                        for (b, c0, c1) in _segments(l):
                            start = not started[b]
                            started[b] = True
                            stop = (l == last_l_for_bank[b]) and (kk == KK - 1)
                            nc.tensor.matmul(
                                ps[b][:, c0 - b * BANK_W:c1 - b * BANK_W],
                                xl[:, kk, :],
                                wg[:, kk - g0, c0 - l * ASL:c1 - l * ASL],
                                start=start, stop=stop,
                            )

            ot = opool.tile([B, OUT_W], mybir.dt.float32)
            for b in range(N_BANKS):
                nc.vector.tensor_copy(ot[:, b * BANK_W:(b + 1) * BANK_W],
                                      ps[b][:])
            nc.sync.dma_start(out_d[:], ot[:])

    nc.compile()
    return nc


def _round_fp32r(a: np.ndarray) -> np.ndarray:
    """Round fp32 to 11 explicit mantissa bits, nearest-even (= HW fp32r)."""
    u = np.ascontiguousarray(a, dtype=np.float32).view(np.uint32)
    low = (u >> np.uint32(12)) & np.uint32(1)
    u = (u + np.uint32(0x7FF) + low) & np.uint32(0xFFFFF000)
    return u.view(np.float32)


def _to_mm(a: np.ndarray) -> np.ndarray:
    if MM_MODE == "f16":
        return np.ascontiguousarray(a).astype(np.float16)
    if MM_MODE == "f32r":
        return _round_fp32r(np.ascontiguousarray(a))
    return np.ascontiguousarray(a, dtype=np.float32)


def _pack_inputs(x: np.ndarray, Ws: list[np.ndarray]):
    """Host-side shard + layout. Returns in_maps (one dict per core)."""
    # xt[l, p, kk, b] = x[b, l, kk*128 + p]  (same for every core)
    xt = _to_mm(x.reshape(B, L, KK, 128).transpose(1, 3, 2, 0))
    # per (l): concat over i >= l of W_i[l]'s per-core 96-col slice
    w_per_l = []
    for l in range(L):
        parts = []
        for i in range(l, L):
            wl = Ws[i][l]  # [F, A]
            parts.append(wl.reshape(KK, 128, N_CORES, ASL)
                           .transpose(2, 1, 0, 3))  # [cores, 128, KK, 96]
        w_per_l.append(np.concatenate(parts, axis=3))  # [cores,128,KK,(L-l)*96]

    in_maps = []
    for j in range(N_CORES):
        m = {"xt": xt}
        for l in range(L):
            m[f"w{l}"] = _to_mm(w_per_l[l][j])
        in_maps.append(m)
    return in_maps


def _run(inputs: dict, trace: bool = False):
    global _compiled_nc
    if _compiled_nc is None:
        _compiled_nc = _build()
    x = np.asarray(inputs["x"], dtype=np.float32)
    Ws = [np.asarray(inputs[f"W_{i}"], dtype=np.float32) for i in range(L)]
    in_maps = _pack_inputs(x, Ws)
    res = run_bass_kernel_spmd(_compiled_nc, in_maps,
                               core_ids=list(range(N_CORES)), trace=trace)
    out = np.empty((B, L, A), dtype=np.float32)
    for j in range(N_CORES):
        out[:, :, j * ASL:(j + 1) * ASL] = res.results[j]["out"].reshape(
            B, L, ASL)
    return out, res


def kernel(**inputs: np.ndarray) -> np.ndarray:
    out, _ = _run(inputs, trace=False)
    return out
